# revision 9
# baseline (speedup 1.0000x reference)
"""Fused LayerNorm + 16-head self-attention + output projection on 8 NeuronCores.

Sharding: core c = (batch b = c//2, head-group g = c%2).  Data parallel over
the 4 batches; tensor parallel over head groups (8 heads each, Megatron-style
column split of W_q/W_kv and row split of W_out).  The two partial outputs
per batch are summed on the host.

Per-core pipeline (matmuls bf16 except the attention O-matmul in fp8):
  A: LayerNorm stats on DVE, apply on GpSimd (xn bf16), PE transposes
     (bf16, 90ns) -> xnt [128, 8, 2048].
  B: projections bf16: qT/kT per head-pair p (w chunks stationary, xnt
     moving); v natural (xnt stationary, wv moving) -> vaug fp8 with a
     ones column at col 64 (cols 65-127 zero).
  C: attention per (qh half, head pair): S^T = kT.T @ qT (K=64, bf16),
     exp on ACT -> fp8 e2 tiles [128, 2kc, 1024]; O^T via fp8 DoubleRow
     matmul (contracts 256 keys/instr): lhsT = vaug [128, 2, 128] gives
     O^T rows 0-63 and the softmax denominator at row 64 in one shot.
     Normalize: DVE reciprocal + gpsimd partition_broadcast + DVE mul
     -> attnt bf16.
  D: out = attnt.T @ W_out (bf16), interleaved into the tail attention
     blocks (queries 0-1023 while the second half is still computing).

Emission is software-pipelined so ACT (exp, the hard ~290us/core floor)
starts ~30us in and never starves: projections for pair p+1, the v
projection, and the first output-projection half run as PE fillers inside
ACT-bound attention blocks.
"""

import numpy as np
import ml_dtypes

import concourse.bacc as bacc
import concourse.tile as tile
from concourse import mybir
from concourse.bass_utils import run_bass_kernel_spmd
from concourse.masks import make_identity

F32 = mybir.dt.float32
BF16 = mybir.dt.bfloat16
FP8 = mybir.dt.float8e4

B, N, D = 4, 2048, 1024
H_TOT, DH, E = 16, 64, 1024
NCORES = 8
HL = 8            # heads per core
EL = HL * DH      # 512 local embed
NT = N // 128     # 16 token tiles
NDC = D // 128    # 8 contraction chunks
NP = 4            # head pairs per core
NKP = NT // 2     # 8 key-chunk pairs
SCALE = float(DH) ** -0.5
EPS = 1e-5

_nc_cache = {}


def _build_nc():
    nc = bacc.Bacc("TRN2", target_bir_lowering=False)
    x = nc.dram_tensor("x", [N, D], BF16, kind="ExternalInput").ap()
    wq = nc.dram_tensor("wq", [D, EL], BF16, kind="ExternalInput").ap()
    wk = nc.dram_tensor("wk", [D, EL], BF16, kind="ExternalInput").ap()
    wv = nc.dram_tensor("wv", [D, EL], BF16, kind="ExternalInput").ap()
    wo = nc.dram_tensor("wo", [EL, D], BF16, kind="ExternalInput").ap()
    out = nc.dram_tensor("out", [N, D], F32, kind="ExternalOutput").ap()

    with tile.TileContext(nc) as tc:
        with (
            tc.tile_pool(name="consts", bufs=1) as consts,
            tc.tile_pool(name="bigsb", bufs=1) as bigsb,
            tc.tile_pool(name="xload", bufs=2) as xload,
            tc.tile_pool(name="xnp", bufs=2) as xnp,
            tc.tile_pool(name="stats", bufs=8) as stats,
            tc.tile_pool(name="wsmall", bufs=16) as wsmall,
            tc.tile_pool(name="e2p", bufs=8) as e2p,
            tc.tile_pool(name="po1", bufs=2, space="PSUM") as po1,
            tc.tile_pool(name="small", bufs=2) as small,
            tc.tile_pool(name="osb", bufs=2) as osbp,
            tc.tile_pool(name="pbig", bufs=2, space="PSUM") as pbig,
        ):
            ident = consts.tile([128, 128], BF16, tag="ident", name="ident")
            make_identity(nc, ident)
            eps_t = consts.tile([128, 1], F32, tag="eps", name="eps")
            nc.vector.memset(eps_t, EPS)

            xnt = bigsb.tile([128, NDC, N], BF16, tag="xnt", name="xnt")
            qt = [
                bigsb.tile([128, N], BF16, tag=f"qt{p}", name=f"qt{p}")
                for p in range(NP)
            ]
            kt = [
                bigsb.tile([128, N], BF16, tag=f"kt{p}", name=f"kt{p}")
                for p in range(NP)
            ]
            attnt = [
                bigsb.tile([128, N], BF16, tag=f"at{p}", name=f"at{p}")
                for p in range(NP)
            ]
            # vaug[:, m, h, 0:64]=v, [.., 64]=1 (ones col -> denominator)
            vaug = bigsb.tile([128, NT, HL, 65], BF16, tag="vaug", name="vaug")
            nc.vector.memset(vaug[:, :, :, 64:65], 1.0)

            wvsb = bigsb.tile([128, NDC, EL], BF16, tag="wvsb", name="wvsb")
            for d in range(NDC):
                nc.sync.dma_start(
                    out=wvsb[:, d, :], in_=wv[d * 128 : (d + 1) * 128, :]
                )
            wosb = bigsb.tile([128, NP, D], BF16, tag="wosb", name="wosb")
            for ec in range(NP):
                nc.sync.dma_start(
                    out=wosb[:, ec, :], in_=wo[ec * 128 : (ec + 1) * 128, :]
                )

            # ---------------- emission helpers --------------------------
            def ln_block(m):
                xt = xload.tile([128, D], BF16, tag="xt", name="xt")
                # column-chunk DMAs land on different queues: lower
                # first-tile latency (each queue is descriptor-rate-bound)
                for cc in range(2):
                    nc.sync.dma_start(
                        out=xt[:, cc * 512 : (cc + 1) * 512],
                        in_=x[m * 128 : (m + 1) * 128, cc * 512 : (cc + 1) * 512],
                    )
                st = stats.tile([128, 2, 6], F32, tag="bn", name="bn")
                nc.vector.bn_stats(out=st[:, 0, :], in_=xt[:, 0:512])
                nc.vector.bn_stats(out=st[:, 1, :], in_=xt[:, 512:1024])
                mv = stats.tile([128, 2], F32, tag="mv", name="mv")
                nc.vector.bn_aggr(out=mv, in_=st)
                sq = stats.tile([128, 1], F32, tag="sq", name="sq")
                nc.scalar.activation(
                    out=sq,
                    in_=mv[:, 1:2],
                    func=mybir.ActivationFunctionType.Sqrt,
                    bias=eps_t,
                    scale=1.0,
                )
                rec = stats.tile([128, 1], F32, tag="rec", name="rec")
                nc.vector.reciprocal(out=rec, in_=sq)
                nmr = stats.tile([128, 1], F32, tag="nmr", name="nmr")
                nc.vector.tensor_scalar(
                    out=nmr,
                    in0=mv[:, 0:1],
                    scalar1=rec,
                    scalar2=-1.0,
                    op0=mybir.AluOpType.mult,
                    op1=mybir.AluOpType.mult,
                )
                xn = xnp.tile([128, D], BF16, tag="xn", name="xn")
                nc.gpsimd.tensor_scalar(
                    out=xn,
                    in0=xt,
                    scalar1=rec,
                    scalar2=nmr,
                    op0=mybir.AluOpType.mult,
                    op1=mybir.AluOpType.add,
                )
                for dp in range(NDC // 2):
                    trp = pbig.tile([128, 2, 128], BF16, tag="big", name="trp")
                    for j in range(2):
                        d = 2 * dp + j
                        nc.tensor.transpose(
                            trp[:, j, :], xn[:, d * 128 : (d + 1) * 128], ident
                        )
                    nc.vector.tensor_copy(
                        out=xnt[:, 2 * dp : 2 * dp + 2, m * 128 : (m + 1) * 128],
                        in_=trp,
                    )

            def qk_proj_quarter(p, w_dram, dst, half):
                """One (weight, token-half) quarter of the q/k projection."""
                wts = []
                for d in range(NDC):
                    wt = wsmall.tile([128, 128], BF16, tag="w", name="w")
                    nc.sync.dma_start(
                        out=wt,
                        in_=w_dram[d * 128 : (d + 1) * 128, p * 128 : (p + 1) * 128],
                    )
                    wts.append(wt)
                pt = pbig.tile([128, 1024], F32, tag="big", name="ptq")
                for d in range(NDC):
                    for ns in range(2):
                        nc.tensor.matmul(
                            out=pt[:, ns * 512 : (ns + 1) * 512],
                            lhsT=wts[d],
                            rhs=xnt[
                                :, d, half * 1024 + ns * 512 : half * 1024 + (ns + 1) * 512
                            ],
                            start=(d == 0),
                            stop=(d == NDC - 1),
                        )
                nc.vector.tensor_copy(
                    out=dst[:, half * 1024 : (half + 1) * 1024], in_=pt
                )

            def qk_proj_fillers(p):
                return [
                    lambda p=p, w=w, dst=dst, half=half: qk_proj_quarter(
                        p, w, dst, half
                    )
                    for half in range(2)
                    for (w, dst) in ((wq, qt[p]), (wk, kt[p]))
                ]

            def v_block(m):
                pv = po1.tile([128, EL], F32, tag="o", name="pv")
                for d in range(NDC):
                    nc.tensor.matmul(
                        out=pv,
                        lhsT=xnt[:, d, m * 128 : (m + 1) * 128],
                        rhs=wvsb[:, d, :],
                        start=(d == 0),
                        stop=(d == NDC - 1),
                    )
                nc.vector.tensor_copy(
                    out=vaug[:, m, :, 0:64],
                    in_=pv.rearrange("p (h dh) -> p h dh", h=HL),
                )

            def outproj_block(m):
                pt = pbig.tile([128, 1024], F32, tag="big", name="pto")
                for ec in range(NP):
                    for ns in range(2):
                        nc.tensor.matmul(
                            out=pt[:, ns * 512 : (ns + 1) * 512],
                            lhsT=attnt[ec][:, m * 128 : (m + 1) * 128],
                            rhs=wosb[:, ec, ns * 512 : (ns + 1) * 512],
                            start=(ec == 0),
                            stop=(ec == NP - 1),
                        )
                ob = osbp.tile([128, D], F32, tag="ob", name="ob")
                nc.vector.tensor_copy(out=ob, in_=pt)
                nc.sync.dma_start(out=out[m * 128 : (m + 1) * 128, :], in_=ob)

            def attention_block(p, qh, fillers=()):
                """S+exp+O for head pair p, query half qh, one head at a time."""
                qoff = qh * 1024
                fillers = list(fillers)
                slot = 0
                for hs in range(2):
                    off = hs * 64
                    e_tiles = {}
                    oacc = None

                    def o_step(kc):
                        for qc in range(2):
                            nc.tensor.matmul(
                                out=oacc[:, qc * 512 : (qc + 1) * 512],
                                lhsT=vaug[:, kc, 2 * p + hs, :],
                                rhs=e_tiles[kc][:, qc * 512 : (qc + 1) * 512],
                                start=(kc == 0),
                                stop=(kc == NT - 1),
                            )

                    for kc in range(NT):
                        stile = pbig.tile([128, 1024], F32, tag="big", name="s")
                        for qc in range(2):
                            nc.tensor.matmul(
                                out=stile[:, qc * 512 : (qc + 1) * 512],
                                lhsT=kt[p][off : off + 64, kc * 128 : (kc + 1) * 128],
                                rhs=qt[p][
                                    off : off + 64,
                                    qoff + qc * 512 : qoff + (qc + 1) * 512,
                                ],
                                start=True,
                                stop=True,
                            )
                        e = e2p.tile([128, 1024], BF16, tag="e2", name="e")
                        nc.scalar.activation(
                            out=e,
                            in_=stile,
                            func=mybir.ActivationFunctionType.Exp,
                            scale=SCALE,
                        )
                        e_tiles[kc] = e
                        if oacc is None:
                            oacc = po1.tile([65, 1024], F32, tag="o", name="oacc")
                        # run O two kc behind its exp so the PE never
                        # waits on a just-issued activation
                        if kc >= 2:
                            o_step(kc - 2)
                        slot += 1
                        if fillers and slot % 7 == 3:
                            fillers.pop(0)()
                    o_step(NT - 2)
                    o_step(NT - 1)
                    while fillers and hs == 1:
                        fillers.pop(0)()
                    # epilogue: normalize rows by the denominator (row 64);
                    # psum row staged through SBUF (approx-recip can't read
                    # PSUM directly)
                    lraw = small.tile([1, 1024], F32, tag="lraw", name="lraw")
                    nc.vector.tensor_copy(out=lraw, in_=oacc[64:65, :])
                    lrow = small.tile([1, 1024], F32, tag="lrow", name="lrow")
                    nc.vector.reciprocal_approx_fast(out=lrow, in_=lraw)
                    lb = small.tile([64, 1024], F32, tag="lb", name="lb")
                    nc.gpsimd.partition_broadcast(lb, lrow)
                    nc.vector.tensor_mul(
                        out=attnt[p][off : off + 64, qoff : qoff + 1024],
                        in0=oacc[0:64, :],
                        in1=lb,
                    )

            # ---------------- emission order ----------------------------
            p0_fillers = qk_proj_fillers(0)
            for m in range(NT):
                ln_block(m)
                v_block(m)
                if m == 7:
                    p0_fillers.pop(0)()   # q half0
                    p0_fillers.pop(0)()   # k half0
            while p0_fillers:
                p0_fillers.pop(0)()

            for qh in range(2):
                for p in range(NP):
                    fillers = []
                    if qh == 0 and p < NP - 1:
                        fillers = qk_proj_fillers(p + 1)
                    elif qh == 1:
                        fillers = [
                            (lambda m=m: outproj_block(m))
                            for m in range(2 * p, 2 * p + 2)
                        ]
                    attention_block(p, qh, fillers=fillers)
            for m in range(8, 16):
                outproj_block(m)

    nc.compile()
    return nc


def _get_nc():
    if "nc" not in _nc_cache:
        _nc_cache["nc"] = _build_nc()
    return _nc_cache["nc"]


def _make_in_maps(q, ln_gamma, ln_beta, W_q, W_kv, W_out):
    q = np.asarray(q, dtype=np.float32)
    g = np.asarray(ln_gamma, dtype=np.float32)
    beta = np.asarray(ln_beta, dtype=np.float32)
    W_q = np.asarray(W_q, dtype=np.float32)
    W_kv = np.asarray(W_kv, dtype=np.float32)
    W_out = np.asarray(W_out, dtype=np.float32)

    assert np.allclose(beta, 0.0, atol=1e-30), (
        "nonzero ln_beta not supported by this kernel build"
    )
    wq_full = (g[:, None] * W_q).astype(ml_dtypes.bfloat16)
    wk_full = (g[:, None] * W_kv[:, :E]).astype(ml_dtypes.bfloat16)
    wv_full = (g[:, None] * W_kv[:, E:]).astype(ml_dtypes.bfloat16)
    wo_full = W_out.astype(ml_dtypes.bfloat16)

    in_maps = []
    for c in range(NCORES):
        b, grp = c // 2, c % 2
        cols = slice(grp * EL, (grp + 1) * EL)
        in_maps.append(
            {
                "x": np.ascontiguousarray(q[b].astype(ml_dtypes.bfloat16)),
                "wq": np.ascontiguousarray(wq_full[:, cols]),
                "wk": np.ascontiguousarray(wk_full[:, cols]),
                "wv": np.ascontiguousarray(wv_full[:, cols]),
                "wo": np.ascontiguousarray(wo_full[cols, :]),
            }
        )
    return in_maps


def _gather(results):
    out = np.empty((B, N, D), dtype=np.float32)
    for b in range(B):
        out[b] = results[2 * b]["out"] + results[2 * b + 1]["out"]
    return out


def kernel(q, ln_gamma, ln_beta, W_q, W_kv, W_out):
    nc = _get_nc()
    in_maps = _make_in_maps(q, ln_gamma, ln_beta, W_q, W_kv, W_out)
    res = run_bass_kernel_spmd(nc, in_maps, core_ids=list(range(NCORES)))
    return _gather(res.results)


def kernel_traced(q, ln_gamma, ln_beta, W_q, W_kv, W_out):
    """Like kernel() but with NTFF profiling; returns (out, BassKernelResults)."""
    nc = _get_nc()
    in_maps = _make_in_maps(q, ln_gamma, ln_beta, W_q, W_kv, W_out)
    res = run_bass_kernel_spmd(nc, in_maps, core_ids=list(range(NCORES)), trace=True)
    return _gather(res.results), res


# revision 10
# speedup vs baseline: 1.1817x; 1.1817x over previous
"""Fused LayerNorm + 16-head self-attention + output projection on 8 NeuronCores.

Sharding: core c = (batch b = c//2, head-group g = c%2).  Data parallel over
the 4 batches; tensor parallel over head groups (8 heads each, Megatron-style
column split of W_q/W_kv and row split of W_out).  The two partial outputs
per batch are summed on the host.

Per-core pipeline (matmuls bf16 except the attention O-matmul in fp8):
  A: LayerNorm stats on DVE, apply on GpSimd (xn bf16), PE transposes
     (bf16, 90ns) -> xnt [128, 8, 2048].
  B: projections bf16: qT/kT per head-pair p (w chunks stationary, xnt
     moving); v natural (xnt stationary, wv moving) -> vaug fp8 with a
     ones column at col 64 (cols 65-127 zero).
  C: attention per (qh half, head pair): S^T = kT.T @ qT (K=64, bf16),
     exp on ACT -> fp8 e2 tiles [128, 2kc, 1024]; O^T via fp8 DoubleRow
     matmul (contracts 256 keys/instr): lhsT = vaug [128, 2, 128] gives
     O^T rows 0-63 and the softmax denominator at row 64 in one shot.
     Normalize: DVE reciprocal + gpsimd partition_broadcast + DVE mul
     -> attnt bf16.
  D: out = attnt.T @ W_out (bf16), interleaved into the tail attention
     blocks (queries 0-1023 while the second half is still computing).

Emission is software-pipelined so ACT (exp, the hard ~290us/core floor)
starts ~30us in and never starves: projections for pair p+1, the v
projection, and the first output-projection half run as PE fillers inside
ACT-bound attention blocks.
"""

import numpy as np
import ml_dtypes

import concourse.bacc as bacc
import concourse.tile as tile
from concourse import mybir
from concourse.bass_utils import run_bass_kernel_spmd
from concourse.masks import make_identity

F32 = mybir.dt.float32
BF16 = mybir.dt.bfloat16
FP8 = mybir.dt.float8e4

B, N, D = 4, 2048, 1024
H_TOT, DH, E = 16, 64, 1024
NCORES = 8
HL = 8            # heads per core
EL = HL * DH      # 512 local embed
NT = N // 128     # 16 token tiles
NDC = D // 128    # 8 contraction chunks
NP = 4            # head pairs per core
NKP = NT // 2     # 8 key-chunk pairs
SCALE = float(DH) ** -0.5
EPS = 1e-5

_nc_cache = {}


def _build_nc():
    nc = bacc.Bacc("TRN2", target_bir_lowering=False)
    x = nc.dram_tensor("x", [N, D], BF16, kind="ExternalInput").ap()
    wq = nc.dram_tensor("wq", [D, EL], BF16, kind="ExternalInput").ap()
    wk = nc.dram_tensor("wk", [D, EL], BF16, kind="ExternalInput").ap()
    wv = nc.dram_tensor("wv", [D, EL], BF16, kind="ExternalInput").ap()
    wo = nc.dram_tensor("wo", [EL, D], BF16, kind="ExternalInput").ap()
    out = nc.dram_tensor("out", [N, D], F32, kind="ExternalOutput").ap()

    with tile.TileContext(nc) as tc:
        with (
            tc.tile_pool(name="consts", bufs=1) as consts,
            tc.tile_pool(name="bigsb", bufs=1) as bigsb,
            tc.tile_pool(name="xload", bufs=2) as xload,
            tc.tile_pool(name="xnp", bufs=2) as xnp,
            tc.tile_pool(name="stats", bufs=8) as stats,
            tc.tile_pool(name="wsmall", bufs=16) as wsmall,
            tc.tile_pool(name="e2p", bufs=8) as e2p,
            tc.tile_pool(name="po1", bufs=2, space="PSUM") as po1,
            tc.tile_pool(name="small", bufs=2) as small,
            tc.tile_pool(name="osb", bufs=2) as osbp,
            tc.tile_pool(name="pbig", bufs=2, space="PSUM") as pbig,
        ):
            ident = consts.tile([128, 128], BF16, tag="ident", name="ident")
            make_identity(nc, ident)
            eps_t = consts.tile([128, 1], F32, tag="eps", name="eps")
            nc.vector.memset(eps_t, EPS)

            xnt = bigsb.tile([128, NDC, N], BF16, tag="xnt", name="xnt")
            qt = [
                bigsb.tile([128, N], BF16, tag=f"qt{p}", name=f"qt{p}")
                for p in range(NP)
            ]
            kt = [
                bigsb.tile([128, N], BF16, tag=f"kt{p}", name=f"kt{p}")
                for p in range(NP)
            ]
            attnt = [
                bigsb.tile([128, N], BF16, tag=f"at{p}", name=f"at{p}")
                for p in range(NP)
            ]
            # vaug[:, m, h, 0:64]=v, [.., 64]=1 (ones col -> denominator)
            vaug = bigsb.tile([128, NT, HL, 65], BF16, tag="vaug", name="vaug")
            nc.vector.memset(vaug[:, :, :, 64:65], 1.0)

            wvsb = bigsb.tile([128, NDC, EL], BF16, tag="wvsb", name="wvsb")
            for d in range(NDC):
                nc.sync.dma_start(
                    out=wvsb[:, d, :], in_=wv[d * 128 : (d + 1) * 128, :]
                )
            wosb = bigsb.tile([128, NP, D], BF16, tag="wosb", name="wosb")
            for ec in range(NP):
                nc.sync.dma_start(
                    out=wosb[:, ec, :], in_=wo[ec * 128 : (ec + 1) * 128, :]
                )

            # ---------------- emission helpers --------------------------
            def ln_block(m):
                xt = xload.tile([128, D], BF16, tag="xt", name="xt")
                # column-chunk DMAs land on different queues: lower
                # first-tile latency (each queue is descriptor-rate-bound)
                for cc in range(2):
                    nc.sync.dma_start(
                        out=xt[:, cc * 512 : (cc + 1) * 512],
                        in_=x[m * 128 : (m + 1) * 128, cc * 512 : (cc + 1) * 512],
                    )
                st = stats.tile([128, 2, 6], F32, tag="bn", name="bn")
                nc.vector.bn_stats(out=st[:, 0, :], in_=xt[:, 0:512])
                nc.vector.bn_stats(out=st[:, 1, :], in_=xt[:, 512:1024])
                mv = stats.tile([128, 2], F32, tag="mv", name="mv")
                nc.vector.bn_aggr(out=mv, in_=st)
                sq = stats.tile([128, 1], F32, tag="sq", name="sq")
                nc.scalar.activation(
                    out=sq,
                    in_=mv[:, 1:2],
                    func=mybir.ActivationFunctionType.Sqrt,
                    bias=eps_t,
                    scale=1.0,
                )
                rec = stats.tile([128, 1], F32, tag="rec", name="rec")
                nc.vector.reciprocal(out=rec, in_=sq)
                nmr = stats.tile([128, 1], F32, tag="nmr", name="nmr")
                nc.vector.tensor_scalar(
                    out=nmr,
                    in0=mv[:, 0:1],
                    scalar1=rec,
                    scalar2=-1.0,
                    op0=mybir.AluOpType.mult,
                    op1=mybir.AluOpType.mult,
                )
                xn = xnp.tile([128, D], BF16, tag="xn", name="xn")
                nc.gpsimd.tensor_scalar(
                    out=xn,
                    in0=xt,
                    scalar1=rec,
                    scalar2=nmr,
                    op0=mybir.AluOpType.mult,
                    op1=mybir.AluOpType.add,
                )
                for dp in range(NDC // 2):
                    trp = pbig.tile([128, 2, 128], BF16, tag="big", name="trp")
                    for j in range(2):
                        d = 2 * dp + j
                        nc.tensor.transpose(
                            trp[:, j, :], xn[:, d * 128 : (d + 1) * 128], ident
                        )
                    nc.scalar.copy(
                        out=xnt[:, 2 * dp : 2 * dp + 2, m * 128 : (m + 1) * 128],
                        in_=trp,
                    )

            def qk_proj_quarter(p, w_dram, dst, half):
                """One (weight, token-half) quarter of the q/k projection."""
                wts = []
                for d in range(NDC):
                    wt = wsmall.tile([128, 128], BF16, tag="w", name="w")
                    nc.sync.dma_start(
                        out=wt,
                        in_=w_dram[d * 128 : (d + 1) * 128, p * 128 : (p + 1) * 128],
                    )
                    wts.append(wt)
                pt = pbig.tile([128, 1024], F32, tag="big", name="ptq")
                for d in range(NDC):
                    for ns in range(2):
                        nc.tensor.matmul(
                            out=pt[:, ns * 512 : (ns + 1) * 512],
                            lhsT=wts[d],
                            rhs=xnt[
                                :, d, half * 1024 + ns * 512 : half * 1024 + (ns + 1) * 512
                            ],
                            start=(d == 0),
                            stop=(d == NDC - 1),
                        )
                nc.vector.tensor_copy(
                    out=dst[:, half * 1024 : (half + 1) * 1024], in_=pt
                )

            def qk_proj_fillers(p):
                return [
                    lambda p=p, w=w, dst=dst, half=half: qk_proj_quarter(
                        p, w, dst, half
                    )
                    for half in range(2)
                    for (w, dst) in ((wq, qt[p]), (wk, kt[p]))
                ]

            def v_mm(m):
                pv = po1.tile([128, EL], F32, tag="o", name="pv")
                for d in range(NDC):
                    nc.tensor.matmul(
                        out=pv,
                        lhsT=xnt[:, d, m * 128 : (m + 1) * 128],
                        rhs=wvsb[:, d, :],
                        start=(d == 0),
                        stop=(d == NDC - 1),
                    )
                return pv

            def v_copy(m, pv):
                nc.scalar.copy(
                    out=vaug[:, m, :, 0:64],
                    in_=pv.rearrange("p (h dh) -> p h dh", h=HL),
                )

            def outproj_block(m):
                pt = pbig.tile([128, 1024], F32, tag="big", name="pto")
                for ec in range(NP):
                    for ns in range(2):
                        nc.tensor.matmul(
                            out=pt[:, ns * 512 : (ns + 1) * 512],
                            lhsT=attnt[ec][:, m * 128 : (m + 1) * 128],
                            rhs=wosb[:, ec, ns * 512 : (ns + 1) * 512],
                            start=(ec == 0),
                            stop=(ec == NP - 1),
                        )
                ob = osbp.tile([128, D], F32, tag="ob", name="ob")
                nc.vector.tensor_copy(out=ob, in_=pt)
                nc.sync.dma_start(out=out[m * 128 : (m + 1) * 128, :], in_=ob)

            def attention_block(p, qh, fillers=()):
                """S+exp+O for head pair p, query half qh, one head at a time."""
                qoff = qh * 1024
                fillers = list(fillers)
                slot = 0
                for hs in range(2):
                    off = hs * 64
                    e_tiles = {}
                    oacc = None

                    def o_step(kc):
                        for qc in range(2):
                            nc.tensor.matmul(
                                out=oacc[:, qc * 512 : (qc + 1) * 512],
                                lhsT=vaug[:, kc, 2 * p + hs, :],
                                rhs=e_tiles[kc][:, qc * 512 : (qc + 1) * 512],
                                start=(kc == 0),
                                stop=(kc == NT - 1),
                            )

                    for kc in range(NT):
                        stile = pbig.tile([128, 1024], F32, tag="big", name="s")
                        for qc in range(2):
                            nc.tensor.matmul(
                                out=stile[:, qc * 512 : (qc + 1) * 512],
                                lhsT=kt[p][off : off + 64, kc * 128 : (kc + 1) * 128],
                                rhs=qt[p][
                                    off : off + 64,
                                    qoff + qc * 512 : qoff + (qc + 1) * 512,
                                ],
                                start=True,
                                stop=True,
                            )
                        e = e2p.tile([128, 1024], BF16, tag="e2", name="e")
                        nc.scalar.activation(
                            out=e,
                            in_=stile,
                            func=mybir.ActivationFunctionType.Exp,
                            scale=SCALE,
                        )
                        e_tiles[kc] = e
                        if oacc is None:
                            oacc = po1.tile([65, 1024], F32, tag="o", name="oacc")
                        # run O two kc behind its exp so the PE never
                        # waits on a just-issued activation
                        if kc >= 2:
                            o_step(kc - 2)
                        slot += 1
                        if fillers and slot % 7 == 3:
                            fillers.pop(0)()
                    o_step(NT - 2)
                    o_step(NT - 1)
                    while fillers and hs == 1:
                        fillers.pop(0)()
                    # epilogue: normalize rows by the denominator (row 64);
                    # psum row staged through SBUF (approx-recip can't read
                    # PSUM directly)
                    lraw = small.tile([1, 1024], F32, tag="lraw", name="lraw")
                    nc.vector.tensor_copy(out=lraw, in_=oacc[64:65, :])
                    lrow = small.tile([1, 1024], F32, tag="lrow", name="lrow")
                    nc.vector.reciprocal_approx_fast(out=lrow, in_=lraw)
                    lb = small.tile([64, 1024], F32, tag="lb", name="lb")
                    nc.gpsimd.partition_broadcast(lb, lrow)
                    nc.vector.tensor_mul(
                        out=attnt[p][off : off + 64, qoff : qoff + 1024],
                        in0=oacc[0:64, :],
                        in1=lb,
                    )

            # ---------------- emission order ----------------------------
            p0_fillers = qk_proj_fillers(0)
            pv_prev = None
            for m in range(NT):
                if pv_prev is not None:
                    pv = v_mm(m - 1)
                ln_block(m)
                if pv_prev is not None:
                    v_copy(m - 1, pv)
                else:
                    pv_prev = True
                if m == 9 or m == 11:
                    p0_fillers.pop(0)()   # q/k half0 (needs m0-7 only)
            pv = v_mm(NT - 1)
            v_copy(NT - 1, pv)
            while p0_fillers:
                p0_fillers.pop(0)()

            for qh in range(2):
                for p in range(NP):
                    fillers = []
                    if qh == 0 and p < NP - 1:
                        fillers = qk_proj_fillers(p + 1)
                    elif qh == 1:
                        fillers = [
                            (lambda m=m: outproj_block(m))
                            for m in range(2 * p, 2 * p + 2)
                        ]
                    attention_block(p, qh, fillers=fillers)
            for m in range(8, 16):
                outproj_block(m)

    nc.compile()
    return nc


def _get_nc():
    if "nc" not in _nc_cache:
        _nc_cache["nc"] = _build_nc()
    return _nc_cache["nc"]


def _make_in_maps(q, ln_gamma, ln_beta, W_q, W_kv, W_out):
    q = np.asarray(q, dtype=np.float32)
    g = np.asarray(ln_gamma, dtype=np.float32)
    beta = np.asarray(ln_beta, dtype=np.float32)
    W_q = np.asarray(W_q, dtype=np.float32)
    W_kv = np.asarray(W_kv, dtype=np.float32)
    W_out = np.asarray(W_out, dtype=np.float32)

    assert np.allclose(beta, 0.0, atol=1e-30), (
        "nonzero ln_beta not supported by this kernel build"
    )
    wq_full = (g[:, None] * W_q).astype(ml_dtypes.bfloat16)
    wk_full = (g[:, None] * W_kv[:, :E]).astype(ml_dtypes.bfloat16)
    wv_full = (g[:, None] * W_kv[:, E:]).astype(ml_dtypes.bfloat16)
    wo_full = W_out.astype(ml_dtypes.bfloat16)

    in_maps = []
    for c in range(NCORES):
        b, grp = c // 2, c % 2
        cols = slice(grp * EL, (grp + 1) * EL)
        in_maps.append(
            {
                "x": np.ascontiguousarray(q[b].astype(ml_dtypes.bfloat16)),
                "wq": np.ascontiguousarray(wq_full[:, cols]),
                "wk": np.ascontiguousarray(wk_full[:, cols]),
                "wv": np.ascontiguousarray(wv_full[:, cols]),
                "wo": np.ascontiguousarray(wo_full[cols, :]),
            }
        )
    return in_maps


def _gather(results):
    out = np.empty((B, N, D), dtype=np.float32)
    for b in range(B):
        out[b] = results[2 * b]["out"] + results[2 * b + 1]["out"]
    return out


def kernel(q, ln_gamma, ln_beta, W_q, W_kv, W_out):
    nc = _get_nc()
    in_maps = _make_in_maps(q, ln_gamma, ln_beta, W_q, W_kv, W_out)
    res = run_bass_kernel_spmd(nc, in_maps, core_ids=list(range(NCORES)))
    return _gather(res.results)


def kernel_traced(q, ln_gamma, ln_beta, W_q, W_kv, W_out):
    """Like kernel() but with NTFF profiling; returns (out, BassKernelResults)."""
    nc = _get_nc()
    in_maps = _make_in_maps(q, ln_gamma, ln_beta, W_q, W_kv, W_out)
    res = run_bass_kernel_spmd(nc, in_maps, core_ids=list(range(NCORES)), trace=True)
    return _gather(res.results), res


# revision 11
# speedup vs baseline: 1.2291x; 1.0401x over previous
"""Fused LayerNorm + 16-head self-attention + output projection on 8 NeuronCores.

Sharding: core c = (batch b = c//2, head-group g = c%2).  Data parallel over
the 4 batches; tensor parallel over head groups (8 heads each, Megatron-style
column split of W_q/W_kv and row split of W_out).  The two partial outputs
per batch are summed on the host.

Per-core pipeline (matmuls bf16 except the attention O-matmul in fp8):
  A: LayerNorm stats on DVE, apply on GpSimd (xn bf16), PE transposes
     (bf16, 90ns) -> xnt [128, 8, 2048].
  B: projections bf16: qT/kT per head-pair p (w chunks stationary, xnt
     moving); v natural (xnt stationary, wv moving) -> vaug fp8 with a
     ones column at col 64 (cols 65-127 zero).
  C: attention per (qh half, head pair): S^T = kT.T @ qT (K=64, bf16),
     exp on ACT -> fp8 e2 tiles [128, 2kc, 1024]; O^T via fp8 DoubleRow
     matmul (contracts 256 keys/instr): lhsT = vaug [128, 2, 128] gives
     O^T rows 0-63 and the softmax denominator at row 64 in one shot.
     Normalize: DVE reciprocal + gpsimd partition_broadcast + DVE mul
     -> attnt bf16.
  D: out = attnt.T @ W_out (bf16), interleaved into the tail attention
     blocks (queries 0-1023 while the second half is still computing).

Emission is software-pipelined so ACT (exp, the hard ~290us/core floor)
starts ~30us in and never starves: projections for pair p+1, the v
projection, and the first output-projection half run as PE fillers inside
ACT-bound attention blocks.
"""

import numpy as np
import ml_dtypes

import concourse.bacc as bacc
import concourse.tile as tile
from concourse import mybir
from concourse.bass_utils import run_bass_kernel_spmd
from concourse.masks import make_identity

F32 = mybir.dt.float32
BF16 = mybir.dt.bfloat16
FP8 = mybir.dt.float8e4

B, N, D = 4, 2048, 1024
H_TOT, DH, E = 16, 64, 1024
NCORES = 8
HL = 8            # heads per core
EL = HL * DH      # 512 local embed
NT = N // 128     # 16 token tiles
NDC = D // 128    # 8 contraction chunks
NP = 4            # head pairs per core
NKP = NT // 2     # 8 key-chunk pairs
SCALE = float(DH) ** -0.5
EPS = 1e-5

_nc_cache = {}


def _build_nc():
    nc = bacc.Bacc("TRN2", target_bir_lowering=False)
    x = nc.dram_tensor("x", [N, D], BF16, kind="ExternalInput").ap()
    wq = nc.dram_tensor("wq", [D, EL], BF16, kind="ExternalInput").ap()
    wk = nc.dram_tensor("wk", [D, EL], BF16, kind="ExternalInput").ap()
    wv = nc.dram_tensor("wv", [D, EL], BF16, kind="ExternalInput").ap()
    wo = nc.dram_tensor("wo", [EL, D], BF16, kind="ExternalInput").ap()
    out = nc.dram_tensor("out", [N, D], F32, kind="ExternalOutput").ap()

    with tile.TileContext(nc) as tc:
        with (
            tc.tile_pool(name="consts", bufs=1) as consts,
            tc.tile_pool(name="bigsb", bufs=1) as bigsb,
            tc.tile_pool(name="xload", bufs=3) as xload,
            tc.tile_pool(name="xnp", bufs=3) as xnp,
            tc.tile_pool(name="stats", bufs=8) as stats,
            tc.tile_pool(name="wsmall", bufs=16) as wsmall,
            tc.tile_pool(name="e2p", bufs=8) as e2p,
            tc.tile_pool(name="po1", bufs=2, space="PSUM") as po1,
            tc.tile_pool(name="small", bufs=2) as small,
            tc.tile_pool(name="osb", bufs=2) as osbp,
            tc.tile_pool(name="pbig", bufs=2, space="PSUM") as pbig,
        ):
            ident = consts.tile([128, 128], BF16, tag="ident", name="ident")
            make_identity(nc, ident)
            eps_t = consts.tile([128, 1], F32, tag="eps", name="eps")
            nc.vector.memset(eps_t, EPS)

            xnt = bigsb.tile([128, NDC, N], BF16, tag="xnt", name="xnt")
            qt = [
                bigsb.tile([128, N], BF16, tag=f"qt{p}", name=f"qt{p}")
                for p in range(NP)
            ]
            kt = [
                bigsb.tile([128, N], BF16, tag=f"kt{p}", name=f"kt{p}")
                for p in range(NP)
            ]
            attnt = [
                bigsb.tile([128, N], BF16, tag=f"at{p}", name=f"at{p}")
                for p in range(NP)
            ]
            # vaug[:, m, h, 0:64]=v, [.., 64]=1 (ones col -> denominator)
            vaug = bigsb.tile([128, NT, HL, 65], BF16, tag="vaug", name="vaug")
            nc.vector.memset(vaug[:, :, :, 64:65], 1.0)

            wvsb = bigsb.tile([128, NDC, EL], BF16, tag="wvsb", name="wvsb")
            for d in range(NDC):
                nc.sync.dma_start(
                    out=wvsb[:, d, :], in_=wv[d * 128 : (d + 1) * 128, :]
                )
            wosb = bigsb.tile([128, NP, D], BF16, tag="wosb", name="wosb")
            for ec in range(NP):
                nc.sync.dma_start(
                    out=wosb[:, ec, :], in_=wo[ec * 128 : (ec + 1) * 128, :]
                )

            # ---------------- emission helpers --------------------------
            def ln_stats(m):
                xt = xload.tile([128, D], BF16, tag="xt", name="xt")
                # column-chunk DMAs land on different queues: lower
                # first-tile latency (each queue is descriptor-rate-bound)
                for cc in range(2):
                    nc.sync.dma_start(
                        out=xt[:, cc * 512 : (cc + 1) * 512],
                        in_=x[m * 128 : (m + 1) * 128, cc * 512 : (cc + 1) * 512],
                    )
                st = stats.tile([128, 2, 6], F32, tag="bn", name="bn")
                nc.vector.bn_stats(out=st[:, 0, :], in_=xt[:, 0:512])
                nc.vector.bn_stats(out=st[:, 1, :], in_=xt[:, 512:1024])
                mv = stats.tile([128, 2], F32, tag="mv", name="mv")
                nc.vector.bn_aggr(out=mv, in_=st)
                sq = stats.tile([128, 1], F32, tag="sq", name="sq")
                nc.scalar.activation(
                    out=sq,
                    in_=mv[:, 1:2],
                    func=mybir.ActivationFunctionType.Sqrt,
                    bias=eps_t,
                    scale=1.0,
                )
                rec = stats.tile([128, 1], F32, tag="rec", name="rec")
                nc.vector.reciprocal(out=rec, in_=sq)
                nmr = stats.tile([128, 1], F32, tag="nmr", name="nmr")
                nc.vector.tensor_scalar(
                    out=nmr,
                    in0=mv[:, 0:1],
                    scalar1=rec,
                    scalar2=-1.0,
                    op0=mybir.AluOpType.mult,
                    op1=mybir.AluOpType.mult,
                )
                xn = xnp.tile([128, D], BF16, tag="xn", name="xn")
                nc.gpsimd.tensor_scalar(
                    out=xn,
                    in0=xt,
                    scalar1=rec,
                    scalar2=nmr,
                    op0=mybir.AluOpType.mult,
                    op1=mybir.AluOpType.add,
                )
                return xn

            def ln_transpose(m, xn):
                for dp in range(NDC // 2):
                    trp = pbig.tile([128, 2, 128], BF16, tag="big", name="trp")
                    for j in range(2):
                        d = 2 * dp + j
                        nc.tensor.transpose(
                            trp[:, j, :], xn[:, d * 128 : (d + 1) * 128], ident
                        )
                    nc.scalar.copy(
                        out=xnt[:, 2 * dp : 2 * dp + 2, m * 128 : (m + 1) * 128],
                        in_=trp,
                    )

            def qk_proj_quarter(p, w_dram, dst, half):
                """One (weight, token-half) quarter of the q/k projection."""
                wts = []
                for d in range(NDC):
                    wt = wsmall.tile([128, 128], BF16, tag="w", name="w")
                    nc.sync.dma_start(
                        out=wt,
                        in_=w_dram[d * 128 : (d + 1) * 128, p * 128 : (p + 1) * 128],
                    )
                    wts.append(wt)
                pt = pbig.tile([128, 1024], F32, tag="big", name="ptq")
                for d in range(NDC):
                    for ns in range(2):
                        nc.tensor.matmul(
                            out=pt[:, ns * 512 : (ns + 1) * 512],
                            lhsT=wts[d],
                            rhs=xnt[
                                :, d, half * 1024 + ns * 512 : half * 1024 + (ns + 1) * 512
                            ],
                            start=(d == 0),
                            stop=(d == NDC - 1),
                        )
                nc.vector.tensor_copy(
                    out=dst[:, half * 1024 : (half + 1) * 1024], in_=pt
                )

            def qk_proj_fillers(p):
                return [
                    lambda p=p, w=w, dst=dst, half=half: qk_proj_quarter(
                        p, w, dst, half
                    )
                    for half in range(2)
                    for (w, dst) in ((wq, qt[p]), (wk, kt[p]))
                ]

            def v_mm(m):
                pv = po1.tile([128, EL], F32, tag="o", name="pv")
                for d in range(NDC):
                    nc.tensor.matmul(
                        out=pv,
                        lhsT=xnt[:, d, m * 128 : (m + 1) * 128],
                        rhs=wvsb[:, d, :],
                        start=(d == 0),
                        stop=(d == NDC - 1),
                    )
                return pv

            def v_copy(m, pv):
                nc.scalar.copy(
                    out=vaug[:, m, :, 0:64],
                    in_=pv.rearrange("p (h dh) -> p h dh", h=HL),
                )

            def outproj_block(m):
                pt = pbig.tile([128, 1024], F32, tag="big", name="pto")
                for ec in range(NP):
                    for ns in range(2):
                        nc.tensor.matmul(
                            out=pt[:, ns * 512 : (ns + 1) * 512],
                            lhsT=attnt[ec][:, m * 128 : (m + 1) * 128],
                            rhs=wosb[:, ec, ns * 512 : (ns + 1) * 512],
                            start=(ec == 0),
                            stop=(ec == NP - 1),
                        )
                ob = osbp.tile([128, D], F32, tag="ob", name="ob")
                nc.vector.tensor_copy(out=ob, in_=pt)
                nc.sync.dma_start(out=out[m * 128 : (m + 1) * 128, :], in_=ob)

            def attention_block(p, qh, fillers=()):
                """S+exp+O for head pair p, query half qh, one head at a time."""
                qoff = qh * 1024
                fillers = list(fillers)
                slot = 0
                for hs in range(2):
                    off = hs * 64
                    e_tiles = {}
                    oacc = None

                    def o_step(kc):
                        for qc in range(2):
                            nc.tensor.matmul(
                                out=oacc[:, qc * 512 : (qc + 1) * 512],
                                lhsT=vaug[:, kc, 2 * p + hs, :],
                                rhs=e_tiles[kc][:, qc * 512 : (qc + 1) * 512],
                                start=(kc == 0),
                                stop=(kc == NT - 1),
                            )

                    for kc in range(NT):
                        stile = pbig.tile([128, 1024], F32, tag="big", name="s")
                        for qc in range(2):
                            nc.tensor.matmul(
                                out=stile[:, qc * 512 : (qc + 1) * 512],
                                lhsT=kt[p][off : off + 64, kc * 128 : (kc + 1) * 128],
                                rhs=qt[p][
                                    off : off + 64,
                                    qoff + qc * 512 : qoff + (qc + 1) * 512,
                                ],
                                start=True,
                                stop=True,
                            )
                        e = e2p.tile([128, 1024], BF16, tag="e2", name="e")
                        nc.scalar.activation(
                            out=e,
                            in_=stile,
                            func=mybir.ActivationFunctionType.Exp,
                            scale=SCALE,
                        )
                        e_tiles[kc] = e
                        if oacc is None:
                            oacc = po1.tile([65, 1024], F32, tag="o", name="oacc")
                        # run O four kc behind its exp so the PE never
                        # waits on a just-issued activation
                        if kc >= 4:
                            o_step(kc - 4)
                        slot += 1
                        if fillers and slot % 7 == 3:
                            fillers.pop(0)()
                    for kc in range(NT - 4, NT):
                        o_step(kc)
                    while fillers and hs == 1:
                        fillers.pop(0)()
                    # epilogue: normalize rows by the denominator (row 64);
                    # psum row staged through SBUF (approx-recip can't read
                    # PSUM directly)
                    lraw = small.tile([1, 1024], F32, tag="lraw", name="lraw")
                    nc.vector.tensor_copy(out=lraw, in_=oacc[64:65, :])
                    lrow = small.tile([1, 1024], F32, tag="lrow", name="lrow")
                    nc.vector.reciprocal_approx_fast(out=lrow, in_=lraw)
                    lb = small.tile([64, 1024], F32, tag="lb", name="lb")
                    nc.gpsimd.partition_broadcast(lb, lrow)
                    nc.vector.tensor_mul(
                        out=attnt[p][off : off + 64, qoff : qoff + 1024],
                        in0=oacc[0:64, :],
                        in1=lb,
                    )

            # ---------------- emission order ----------------------------
            p0_fillers = qk_proj_fillers(0)
            xns = {}
            for m in range(NT + 2):
                if m < NT:
                    xns[m] = ln_stats(m)
                if m >= 1 and m - 1 < NT:
                    ln_transpose(m - 1, xns.pop(m - 1))
                if m >= 2:
                    pv = v_mm(m - 2)
                    v_copy(m - 2, pv)
                if m == 10 or m == 12:
                    p0_fillers.pop(0)()   # q/k half0 (needs m0-7 only)
            while p0_fillers:
                p0_fillers.pop(0)()

            for qh in range(2):
                for p in range(NP):
                    fillers = []
                    if qh == 0 and p < NP - 1:
                        fillers = qk_proj_fillers(p + 1)
                    elif qh == 1:
                        fillers = [
                            (lambda m=m: outproj_block(m))
                            for m in range(2 * p, 2 * p + 2)
                        ]
                    attention_block(p, qh, fillers=fillers)
            for m in range(8, 16):
                outproj_block(m)

    nc.compile()
    return nc


def _get_nc():
    if "nc" not in _nc_cache:
        _nc_cache["nc"] = _build_nc()
    return _nc_cache["nc"]


def _make_in_maps(q, ln_gamma, ln_beta, W_q, W_kv, W_out):
    q = np.asarray(q, dtype=np.float32)
    g = np.asarray(ln_gamma, dtype=np.float32)
    beta = np.asarray(ln_beta, dtype=np.float32)
    W_q = np.asarray(W_q, dtype=np.float32)
    W_kv = np.asarray(W_kv, dtype=np.float32)
    W_out = np.asarray(W_out, dtype=np.float32)

    assert np.allclose(beta, 0.0, atol=1e-30), (
        "nonzero ln_beta not supported by this kernel build"
    )
    wq_full = (g[:, None] * W_q).astype(ml_dtypes.bfloat16)
    wk_full = (g[:, None] * W_kv[:, :E]).astype(ml_dtypes.bfloat16)
    wv_full = (g[:, None] * W_kv[:, E:]).astype(ml_dtypes.bfloat16)
    wo_full = W_out.astype(ml_dtypes.bfloat16)

    in_maps = []
    for c in range(NCORES):
        b, grp = c // 2, c % 2
        cols = slice(grp * EL, (grp + 1) * EL)
        in_maps.append(
            {
                "x": np.ascontiguousarray(q[b].astype(ml_dtypes.bfloat16)),
                "wq": np.ascontiguousarray(wq_full[:, cols]),
                "wk": np.ascontiguousarray(wk_full[:, cols]),
                "wv": np.ascontiguousarray(wv_full[:, cols]),
                "wo": np.ascontiguousarray(wo_full[cols, :]),
            }
        )
    return in_maps


def _gather(results):
    out = np.empty((B, N, D), dtype=np.float32)
    for b in range(B):
        out[b] = results[2 * b]["out"] + results[2 * b + 1]["out"]
    return out


def kernel(q, ln_gamma, ln_beta, W_q, W_kv, W_out):
    nc = _get_nc()
    in_maps = _make_in_maps(q, ln_gamma, ln_beta, W_q, W_kv, W_out)
    res = run_bass_kernel_spmd(nc, in_maps, core_ids=list(range(NCORES)))
    return _gather(res.results)


def kernel_traced(q, ln_gamma, ln_beta, W_q, W_kv, W_out):
    """Like kernel() but with NTFF profiling; returns (out, BassKernelResults)."""
    nc = _get_nc()
    in_maps = _make_in_maps(q, ln_gamma, ln_beta, W_q, W_kv, W_out)
    res = run_bass_kernel_spmd(nc, in_maps, core_ids=list(range(NCORES)), trace=True)
    return _gather(res.results), res
